# revision 14
# baseline (speedup 1.0000x reference)
"""Trainium2 Bass kernel for nn_CCFLoss (masked-MSE heat/offset losses + argmax-gathered
class-balanced BCE), data-parallel over batch across 8 NeuronCores.

v9: the three masked-MSE sums are computed as Frobenius inner products
    sum((p-t)*w)^2 = <d^2, w^2> = trace((d^2)^T (w^2))
so the DVE only does subtractions (plus the exact f32 argmax scan), the ACT
engine does the elementwise squares, and the otherwise-idle TensorE contracts
everything into a single accumulating [128,128] PSUM bank whose diagonal the
host sums. GPSIMD is left idle on purpose: it shares an SBUF port with the
DVE, and any GPSIMD op measurably stretches concurrent DVE ops ~4x.

HBM traffic: 14 B/elem (ht f32 for the exact argmax tie-break, heat_pred+mask
as fp8e3 - they are only read by 1x-rate ops whose cost is dtype-independent -
offsets as bf16), vs 28 B/elem for the all-f32 baseline. All 7 tensors of a
group are host-packed into ONE contiguous byte block per group ([128, 28672]
u8 = 28 KB/partition rows), DMAed in a single descriptor-friendly transfer,
and read on-device through bitcast views.

Layout per core (2 batches = 22 images): 5 groups of 4 images as [32, 2048]
blocks stacked on partitions -> [128, 2048] tiles, plus one tail group of 2
images as [64, 1024] -> [128, 1024], emitted second-to-last so the pipeline
drains on a big, well-overlapped group. dh^2 runs on ACT for 4 of the 5 big
groups and on DVE otherwise, balancing those engines at ~52 us each.

Host: sums diag / n_el for the MSE part, picks the global argmax per (b,c)
from per-partition top-1s, gathers clss_* at those 176 locations, finishes the
masked BCE means on scalars in float64.
"""
import sys

if "/opt/trn_rl_repo" not in sys.path:
    sys.path.insert(0, "/opt/trn_rl_repo")

import numpy as np
import ml_dtypes

B, C, H, W = 16, 11, 256, 256
P = 128
NCORES = 8
BPC = B // NCORES          # batches per core
NPAIR = BPC * C            # images per core (22)
N_V_CHANNELS = 5
# emission order: B A0 A1 A2 A3 A4 -> (gidx, n_images, img_partitions, fd);
# the small group goes first so the pipeline fills fast, and the last (big)
# group drains while fully overlapped
GORDER = [(5, 2, 64, 1024), (0, 4, 32, 2048), (1, 4, 32, 2048),
          (2, 4, 32, 2048), (3, 4, 32, 2048), (4, 4, 32, 2048)]
ROW_A, ROW_B = 28672, 14336   # mega-row bytes/partition (A: fd=2048, B: 1024)

_STATE = {}


def _pos_weight(samples):
    s = np.asarray(samples, dtype=np.float64)
    beta = (s - 1.0) / s
    en = (1.0 - np.power(beta, s)) / (1.0 - beta)
    w = 1.0 / (en + 1e-5)
    return float(w[1] / (w[0] + 1e-5))


POS_W_V = _pos_weight([8000.0, 2000.0])
POS_W_D = _pos_weight([7000.0, 2000.0 + 1000.0])


def _build():
    import concourse.bacc as bacc
    import concourse.tile as tile
    import concourse.mybir as mybir

    f32 = mybir.dt.float32
    bf16 = mybir.dt.bfloat16
    fp8 = mybir.dt.float8e3
    u8 = mybir.dt.uint8
    u32 = mybir.dt.uint32
    SQUARE = mybir.ActivationFunctionType.Square

    nc = bacc.Bacc("TRN2", target_bir_lowering=False, debug=False)
    megaA = nc.dram_tensor("megaA", [5, P, ROW_A], u8, kind="ExternalInput").ap()
    megaB = nc.dram_tensor("megaB", [P, ROW_B], u8, kind="ExternalInput").ap()
    diag_d = nc.dram_tensor("diag", [P, P], f32, kind="ExternalOutput").ap()
    vals_d = nc.dram_tensor("vals8", [P, 8 * 6], f32, kind="ExternalOutput").ap()
    idx_d = nc.dram_tensor("idx8", [P, 8 * 6], u32, kind="ExternalOutput").ap()

    n_mm = sum(3 * (fd // 128) for _, _, _, fd in GORDER)
    with tile.TileContext(nc) as tc:
        with tc.tile_pool(name="ins", bufs=4) as ipool, \
             tc.tile_pool(name="work", bufs=2) as wpool, \
             tc.tile_pool(name="acc", bufs=1) as apool, \
             tc.tile_pool(name="ps", bufs=1, space="PSUM") as pspool:
            vals_t = apool.tile([P, 8 * 6], f32)
            idx_t = apool.tile([P, 8 * 6], u32)
            psum_t = pspool.tile([P, P], f32)

            mm_i = 0
            # prefetch the ht sections of the first three groups so the DVE
            # argmax scans never starve during pipeline fill
            pre = []
            for order_i in range(3):
                g, nimg, pi, fd = GORDER[order_i]
                srcg = megaA[g] if fd == 2048 else megaB
                mega = ipool.tile([P, ROW_A], u8, tag="mega")
                eng = nc.scalar if order_i == 0 else nc.sync
                eng.dma_start(out=mega[:, :4 * fd], in_=srcg[:, :4 * fd])
                pre.append(mega)
            for order_i, (g, nimg, pi, fd) in enumerate(GORDER):
                row = ROW_A if fd == 2048 else ROW_B
                src = megaA[g] if fd == 2048 else megaB
                if order_i < 3:
                    mega = pre[order_i]
                else:
                    mega = ipool.tile([P, ROW_A], u8, tag="mega")
                    nc.sync.dma_start(out=mega[:, :4 * fd],
                                      in_=src[:, :4 * fd])
                # remaining sections: finer arrival granularity so consumers
                # start as sections land
                eng2 = nc.scalar if order_i == 0 else nc.sync
                eng2.dma_start(out=mega[:, 4 * fd:6 * fd],
                               in_=src[:, 4 * fd:6 * fd])
                nc.sync.dma_start(out=mega[:, 6 * fd:10 * fd],
                                  in_=src[:, 6 * fd:10 * fd])
                eng2.dma_start(out=mega[:, 10 * fd:14 * fd],
                               in_=src[:, 10 * fd:14 * fd])
                # bitcast views into the packed row:
                #   ht f32 | hp fp8 | m fp8 | oyp,oxp bf16 | oyt,oxt bf16
                ht = mega[:, :4 * fd].bitcast(f32)
                hp = mega[:, 4 * fd:5 * fd].bitcast(fp8)
                m_ = mega[:, 5 * fd:6 * fd].bitcast(fp8)
                oyxp = mega[:, 6 * fd:10 * fd].bitcast(bf16)
                oyxt = mega[:, 10 * fd:14 * fd].bitcast(bf16)

                # per-partition top-8 of ht (f32, exact) - covers all images
                v8 = vals_t[:, 8 * g:8 * g + 8]
                nc.vector.max(out=v8, in_=ht)
                nc.vector.max_index(out=idx_t[:, 8 * g:8 * g + 8],
                                    in_max=v8, in_values=ht)

                # squares of the weights (ACT, 1x rate, any input dtype)
                ht2_t = wpool.tile([P, 2048], bf16, tag="ht2")
                ht2 = ht2_t[:, :fd]
                nc.scalar.activation(ht2, ht, SQUARE)
                m2_t = wpool.tile([P, 2048], bf16, tag="m2")
                m2 = m2_t[:, :fd]
                nc.scalar.activation(m2, m_, SQUARE)

                # diffs (DVE) and their squares (DVE/ACT balanced)
                dh_t = wpool.tile([P, 2048], bf16, tag="dh")
                dh = dh_t[:, :fd]
                nc.vector.tensor_sub(out=dh, in0=hp, in1=ht)
                dh2_t = wpool.tile([P, 2048], bf16, tag="dh2")
                dh2 = dh2_t[:, :fd]
                if order_i <= 1:
                    nc.vector.tensor_mul(out=dh2, in0=dh, in1=dh)
                else:
                    nc.scalar.activation(dh2, dh, SQUARE)
                dyx_t = wpool.tile([P, 4096], bf16, tag="dyx")
                dyx = dyx_t[:, :2 * fd]
                nc.vector.tensor_sub(out=dyx, in0=oyxp, in1=oyxt)
                dyx2_t = wpool.tile([P, 4096], bf16, tag="dyx2")
                dyx2 = dyx2_t[:, :2 * fd]
                if order_i == len(GORDER) - 1:
                    nc.scalar.activation(dyx2_t[:, :fd], dyx_t[:, :fd], SQUARE)
                    nc.scalar.activation(dyx2_t[:, fd:2 * fd],
                                         dyx_t[:, fd:2 * fd], SQUARE)
                else:
                    nc.scalar.activation(dyx2, dyx, SQUARE)

                # accumulate sum(d^2 * w^2) = trace((d^2)^T (w^2)) chunkwise
                # into one PSUM bank; host reads the diagonal. lhsT=weight^2 so
                # the off-term reuses each ht^2 chunk for both dy and dx.
                nch = fd // 128
                for c in range(nch):
                    s = slice(128 * c, 128 * c + 128)
                    nc.tensor.matmul(psum_t[:], lhsT=m2[:, s], rhs=dh2[:, s],
                                     start=(mm_i == 0), stop=(mm_i == n_mm - 1))
                    mm_i += 1
                    nc.tensor.matmul(psum_t[:], lhsT=ht2[:, s], rhs=dyx2[:, s],
                                     start=False, stop=(mm_i == n_mm - 1))
                    mm_i += 1
                    s2 = slice(fd + 128 * c, fd + 128 * c + 128)
                    nc.tensor.matmul(psum_t[:], lhsT=ht2[:, s], rhs=dyx2[:, s2],
                                     start=False, stop=(mm_i == n_mm - 1))
                    mm_i += 1

            diag_s = apool.tile([P, P], f32)
            nc.scalar.copy(out=diag_s[:], in_=psum_t[:])
            nc.sync.dma_start(out=diag_d, in_=diag_s[:])
            nc.sync.dma_start(out=vals_d, in_=vals_t[:])
            nc.sync.dma_start(out=idx_d, in_=idx_t[:])

    nc.compile()
    return nc


def _get_nc():
    if "nc" not in _STATE:
        _STATE["nc"] = _build()
    return _STATE["nc"]


def _softplus(x):
    return np.log1p(np.exp(-np.abs(x))) + np.maximum(x, 0.0)


def run_device(in_maps, **kwargs):
    from concourse.bass_utils import run_bass_kernel_spmd
    nc = _get_nc()
    return run_bass_kernel_spmd(nc, in_maps, core_ids=list(range(NCORES)), **kwargs)


def make_in_maps(inp):
    fp8 = ml_dtypes.float8_e3m4
    bf16 = ml_dtypes.bfloat16
    names = ("ht", "hp", "m", "oyp", "oxp", "oyt", "oxt")
    src = {"ht": inp["heat_targets"], "hp": inp["heat_predictions"],
           "m": inp["masks"], "oyp": inp["offy_predictions"],
           "oxp": inp["offx_predictions"], "oyt": inp["offy_targets"],
           "oxt": inp["offx_targets"]}
    # -> [NCORES, NPAIR, H*W] f32
    src = {n: np.ascontiguousarray(a, dtype=np.float32)
           .reshape(NCORES, NPAIR, H * W) for n, a in src.items()}

    def pack(pieces):  # list of [ng, P, fd]-shaped blocks -> packed u8 rows
        return np.ascontiguousarray(np.concatenate(
            [p.view(np.uint8).reshape(p.shape[0], P, -1) for p in pieces],
            axis=-1))

    in_maps = []
    for k in range(NCORES):
        im = {}
        for cls, sl, fd in (("A", slice(0, 20), 2048), ("B", slice(20, 22), 1024)):
            blk = {n: src[n][k, sl].reshape(-1, P, fd) for n in names}
            mega = pack([blk["ht"],
                         blk["hp"].astype(fp8), blk["m"].astype(fp8),
                         blk["oyp"].astype(bf16), blk["oxp"].astype(bf16),
                         blk["oyt"].astype(bf16), blk["oxt"].astype(bf16)])
            im["mega" + cls] = mega if cls == "A" else mega[0]
        in_maps.append(im)
    return in_maps


def finish_host(results, inp):
    """Combine per-core device outputs into the final scalar loss (float64 host math)."""
    cp = np.asarray(inp["clss_predictions"], dtype=np.float32).reshape(B, C, H * W)
    ct = np.asarray(inp["clss_targets"], dtype=np.float32).reshape(B, C, H * W)
    v_w = float(np.asarray(inp["v_loss_weight"]))
    d_w = float(np.asarray(inp["d_loss_weight"]))

    mse_sum = 0.0
    g_pred = np.zeros((B, C), dtype=np.float64)
    g_tgt = np.zeros((B, C), dtype=np.float64)
    groups = [(4, 32, 2048)] * 5 + [(2, 64, 1024)]   # in image order
    for k in range(NCORES):
        out = results[k]
        mse_sum += float(np.trace(np.asarray(out["diag"], dtype=np.float64)))
        pm = np.asarray(out["vals8"]).reshape(P, 6, 8)[:, :, 0]
        ji = np.asarray(out["idx8"]).reshape(P, 6, 8)[:, :, 0]
        i = 0
        for g, (nimg, pi, fd) in enumerate(groups):
            for h in range(nimg):
                b = k * BPC + i // C
                c = i % C
                p_star = int(np.argmax(pm[pi * h:pi * h + pi, g]))
                flat = p_star * fd + int(ji[pi * h + p_star, g])
                g_pred[b, c] = cp[b, c, flat]
                g_tgt[b, c] = ct[b, c, flat]
                i += 1

    n_el = float(B * C * H * W)
    mse_loss = mse_sum / n_el   # heat + offy + offx (all weights are 1.0)

    valid = g_tgt >= 0.0
    is_v = (np.arange(C) < N_V_CHANNELS)[None, :]
    v_mask = (valid & is_v).astype(np.float64)
    d_mask = (valid & ~is_v).astype(np.float64)

    x = g_pred
    sp_neg = _softplus(-x)
    sp_pos = _softplus(x)

    l_v = POS_W_V * g_tgt * sp_neg + (1.0 - g_tgt) * sp_pos
    v_cls = (l_v * v_mask).sum() / max(v_mask.sum(), 1.0)
    y_d = (g_tgt >= 1.0).astype(np.float64)
    l_d = POS_W_D * y_d * sp_neg + (1.0 - y_d) * sp_pos
    d_cls = (l_d * d_mask).sum() / max(d_mask.sum(), 1.0)

    loss = mse_loss + v_cls * v_w + d_cls * d_w
    return np.float32(loss)


def kernel(**inputs):
    inp = {k: np.asarray(v) for k, v in inputs.items()}
    in_maps = make_in_maps(inp)
    res = run_device(in_maps)
    return finish_host(res.results, inp)
